# revision 45
# baseline (speedup 1.0000x reference)
"""Multi-head causal attention (B=4, S=2048, D=1024, 16 heads) on 8 TRN2 cores.

Sharding: core c -> (batch b = c//2, head-group g = c%2). Each core computes
8 heads of one batch element end-to-end (QKV proj, causal softmax attention,
out-proj rows for its head slice). Host sums the two head-group partials per
batch and adds the output bias.

Per-core pipeline (all matmuls contraction-on-partitions, bf16 in / f32 psum):
  V first (quarter 0 din-major so each (wv,xT-chunk) DMA pair unblocks work;
  later quarters group-major so psum->sbuf copies overlap the remaining
  matmuls), then QT/KT[dtile] = (x @ w)^T with attention interleaved.
  attention per (256-wide q-chunk qc, head-pair hp):
    ST[k,q] = KT.T @ QT into a [128,1024] psum (<=4 k-blocks per exp chunk)
    PT = exp(ST/8) bf16; tri-mask on diagonal 128-col blocks
    ctxT[q128, 65] += PT-block.T @ [V_h|1]   (N=65 matmuls: cost-model cheap;
      col 64 = softmax denominator). One start=True per ctx psum bank (PSUM
      pending-zero covers every block's first write; a second start would
      wipe sibling blocks).  PV trails the exp by 2 chunk-halves ACROSS
      attention units so the PE never sits behind the ACT engine.
    normalize per-partition: ctxT[:, :64] * recip(ctxT[:, 64]) -> t_sb bf16
  t_sb tiles are PE-transposed (via identity) back to cxt[2-head-hd, q].
  out[seq128, 512] = cxt.T @ ow streamed to DRAM per 128-row block.
  Transposes + out-proj pieces are "fillers" popped between score chunks;
  the late phase runs qc descending so out-proj work for early rows is
  available to fill the exp-heavy qc=7/6 units.
"""

import numpy as np
import ml_dtypes

B, S, D = 4, 2048, 1024
H_TOT = 16
HD = 64
NCORES = 8
GH = 8          # heads per core
GD = GH * HD    # 512: dout slice per core
NKB = S // 128  # 16 k-blocks
QW = 256        # q-chunk width
NQC = S // QW   # 8 q-chunks
BF16 = ml_dtypes.bfloat16

PACK_HEADS = True   # pack 2 heads' score matmuls into PE row groups
PV_DEPTH = 3        # chunk-halves the PV trails behind the exp
AIW = 512 * 3 + S + 1024   # packed input width: [wv | wq | wk | xT | ow]

_cache = {}


def _build_body(tc, nc, mybir, allin, outp, dbg=None):
    from concourse.masks import make_upper_triangular, make_identity
    import contextlib

    dt = mybir.dt
    F = mybir.ActivationFunctionType

    pools = contextlib.ExitStack()
    tc_pool = lambda **kw: pools.enter_context(tc.tile_pool(**kw))

    singles = tc_pool(name="singles", bufs=1)
    pt_pool = tc_pool(name="pt", bufs=6)
    tsb_pool = tc_pool(name="tsb", bufs=8)
    small = tc_pool(name="small", bufs=4)
    ost_pool = tc_pool(name="ost", bufs=4)
    psum_st = tc_pool(name="psum_st", bufs=2, space="PSUM")
    psum_ctx = tc_pool(name="psum_ctx", bufs=2, space="PSUM")
    psum_mm = tc_pool(name="psum_mm", bufs=2, space="PSUM")

    # ---- persistent SBUF tensors.  All per-din inputs live in ONE packed
    # tile per din ([wv | xtA | wq | wk | xtB | ow]) so the whole input load
    # is 24 large DMAs: HWDGE descriptor generation (~0.6us per DMA, one
    # shared unit) would otherwise serialize ~60 small DMAs for ~38us.  The
    # V-projection operands come first so the PE starts ~2us in.
    allin_sb = [singles.tile([128, AIW], dt.bfloat16, name=f"ai{t}")
                for t in range(8)]
    wv_sb = [t[:, 0:512] for t in allin_sb]
    wq_sb = [t[:, 1536:2048] for t in allin_sb]
    wk_sb = [t[:, 2048:2560] for t in allin_sb]
    ow_sb = [allin_sb[t][:, 3584:AIW] for t in range(4)]

    def xts(din, s0, s1):
        """xT[din] seq-col slice [s0:s1); must not cross the 1024 boundary."""
        if s1 <= 1024:
            return allin_sb[din][:, 512 + s0:512 + s1]
        assert s0 >= 1024
        return allin_sb[din][:, 2560 + s0 - 1024:2560 + s1 - 1024]

    xT_sb = None  # accessed via xts()
    qt_sb = [singles.tile([128, S], dt.bfloat16, name=f"qt{t}")
             for t in range(4)]                              # 2 heads / dtile
    kt_sb = [singles.tile([128, S], dt.bfloat16, name=f"kt{t}")
             for t in range(4)]
    vo_sb = [singles.tile([128, GH, 65], dt.bfloat16, name=f"vo{t}")
             for t in range(NKB)]                            # [V_h | ones]
    cxt_sb = [singles.tile([128, S], dt.bfloat16, name=f"cxt{t}")
              for t in range(4)]                             # ctx, 2 heads/tile
    tri = singles.tile([128, 128], dt.bfloat16)              # keep k<=q
    ident = singles.tile([128, 128], dt.bfloat16)

    make_upper_triangular(nc, tri, val=1.0, diag=True)
    make_identity(nc, ident)
    for t in range(NKB):
        nc.vector.memset(vo_sb[t][:, :, 64:65], 1.0)

    # ---- input DMAs: 3 pieces per din on the SP HWDGE queue.  Piece A
    # (wv + xT first half) feeds the V projections, B (wq+wk) the Q/K
    # projections, C the rest.  The ACT queue carries nothing, so the ACT
    # sequencer is free for exp dispatch.
    ai_r = allin.ap().rearrange("(t p) n -> p t n", p=128)
    for lo, hi in ((0, 1536), (1536, 2560), (2560, AIW)):
        for din in range(8):
            nc.sync.dma_start(out=allin_sb[din][:, lo:hi],
                              in_=ai_r[:, din, lo:hi])

    def emit_v_quarter(q4):
        """V proj for seq blocks 4*q4..4*q4+3.  Quarter 0 is DMA-paced:
        din-major so each arriving (wv,xT) pair feeds 4 matmuls.  Later
        quarters are group-major so each group's copy overlaps the rest."""
        pst = [psum_st.tile([128, 1024], dt.float32, name="stp")
               for _ in range(2)]
        pss = [pst[0][:, 0:512], pst[0][:, 512:1024],
               pst[1][:, 0:512], pst[1][:, 512:1024]]

        def mm(i, din):
            st = 4 * q4 + i
            nc.tensor.matmul(
                pss[i],
                lhsT=xts(din, st * 128, (st + 1) * 128),
                rhs=wv_sb[din],
                start=(din == 0),
                stop=(din == 7),
            )

        def cp(i):
            nc.vector.tensor_copy(
                out=vo_sb[4 * q4 + i][:, :, 0:64],
                in_=pss[i].rearrange("p (h d) -> p h d", h=GH),
            )

        if q4 == 0:
            for din in range(8):
                for i in range(4):
                    mm(i, din)
            for i in range(4):
                cp(i)
        else:
            for i in range(4):
                for din in range(8):
                    mm(i, din)
                cp(i)

    def emit_v_st(st):
        """One V seq-block as late-phase filler work (psum_mm based)."""
        ps = psum_mm.tile([128, 512], dt.float32, name="mmps")
        for din in range(8):
            nc.tensor.matmul(
                ps,
                lhsT=xts(din, st * 128, (st + 1) * 128),
                rhs=wv_sb[din],
                start=(din == 0),
                stop=(din == 7),
            )
        nc.vector.tensor_copy(
            out=vo_sb[st][:, :, 0:64],
            in_=ps.rearrange("p (h d) -> p h d", h=GH),
        )
        v_emitted.add(st)

    def emit_proj_pair(d, c):
        """Q and K projection of dtile d, 512-col seq chunk c (one stp tile)."""
        pst = psum_st.tile([128, 1024], dt.float32, name="stp")
        for half, (w_sb, t_sb) in enumerate(((wq_sb, qt_sb), (wk_sb, kt_sb))):
            ps = pst[:, half * 512:(half + 1) * 512]
            for din in range(8):
                nc.tensor.matmul(
                    ps,
                    lhsT=w_sb[din][:, d * 128:(d + 1) * 128],
                    rhs=xts(din, c * 512, (c + 1) * 512),
                    start=(din == 0),
                    stop=(din == 7),
                )
            nc.vector.tensor_copy(
                out=t_sb[d][:, c * 512:(c + 1) * 512], in_=ps)

    def emit_proj_half(d, c, half):
        """Half a proj pair (Q or K) as late filler work (psum_mm based)."""
        w_sb, t_sb = ((wq_sb, qt_sb), (wk_sb, kt_sb))[half]
        ps = psum_mm.tile([128, 512], dt.float32, name="mmps")
        for din in range(8):
            nc.tensor.matmul(
                ps,
                lhsT=w_sb[din][:, d * 128:(d + 1) * 128],
                rhs=xts(din, c * 512, (c + 1) * 512),
                start=(din == 0),
                stop=(din == 7),
            )
        nc.vector.tensor_copy(
            out=t_sb[d][:, c * 512:(c + 1) * 512], in_=ps)

    # ---- filler machinery: small PE work units popped between score chunks.
    # Pops are RATIONED to ~the per-chunk-half ACT-vs-PE deficit so the
    # supply lasts through the final (most exp-heavy) units.
    LATE0 = 56          # chunk-half index where the ACT-bound phase begins
    DEF_NS = 800        # filler allowance earned per chunk-half
    slot = [0]          # global chunk-half counter
    allow = [0.0]       # accumulated filler allowance (ns)
    fillers = []        # [prio, ready_slot, cost_ns, fn] entries
    tr_done = [0] * NQC
    v_emitted = set(range(8))   # V seq-blocks already materialized

    def emit_filler():
        allow[0] += DEF_NS
        while True:
            best = None
            for i, (prio, rdy, cost, fn) in enumerate(fillers):
                if rdy <= slot[0] and cost <= allow[0] and (
                        best is None or prio < fillers[best][0]):
                    best = i
            if best is None:
                return
            ent = fillers.pop(best)
            allow[0] -= ent[2]
            ent[3]()

    def force_v(st_max):
        """Emit any still-queued V fillers with st <= st_max right now."""
        for i in range(len(fillers) - 1, -1, -1):
            if getattr(fillers[i][3], "v_st", 99) <= st_max:
                fillers.pop(i)[3]()

    def force_p3(d):
        """Emit any still-queued chunk-3 proj fillers for dtile d."""
        for i in range(len(fillers) - 1, -1, -1):
            if getattr(fillers[i][3], "p3_d", -1) == d:
                fillers.pop(i)[3]()

    def flush_fillers():
        while fillers:
            best = None
            for i, ent in enumerate(fillers):
                if best is None or ent[0] < fillers[best][0]:
                    best = i
            fillers.pop(best)[3]()

    def emit_transpose(qc, hp):
        tps = psum_mm.tile([128, 256], dt.bfloat16, name="mmps")
        ts0, ts1 = tsb_tiles.pop((qc, hp))
        nc.tensor.transpose(tps[:, 0:128], ts0, ident)
        nc.tensor.transpose(tps[:, 128:256], ts1, ident)
        nc.vector.tensor_copy(
            out=cxt_sb[hp][:, qc * QW:(qc + 1) * QW], in_=tps)
        tr_done[qc] += 1
        if tr_done[qc] == 4:
            for sq in (2 * qc, 2 * qc + 1):
                for oc in range(2):
                    fillers.append(
                        [2, max(slot[0] + 2, LATE0), 900,
                         lambda sq=sq, oc=oc: emit_outproj_piece(sq, oc)])

    ost_tiles = {}

    def emit_outproj_piece(sq, oc):
        ps = psum_mm.tile([128, 512], dt.float32, name="mmps")
        for dvt in range(4):
            nc.tensor.matmul(
                ps,
                lhsT=cxt_sb[dvt][:, sq * 128:(sq + 1) * 128],
                rhs=ow_sb[dvt][:, oc * 512:(oc + 1) * 512],
                start=(dvt == 0),
                stop=(dvt == 3),
            )
        if sq // 2 == 2:    # final range (qc=2 is last): fine-grained drain
            ost = ost_pool.tile([128, 1024], dt.bfloat16, name="ost")
            for i in range(2):
                nc.vector.tensor_copy(
                    out=ost[:, i * 256:(i + 1) * 256],
                    in_=ps[:, i * 256:(i + 1) * 256])
                nc.sync.dma_start(
                    out=outp.ap()[sq * 128:(sq + 1) * 128,
                                  oc * 512 + i * 256:oc * 512 + (i + 1) * 256],
                    in_=ost[:, i * 256:(i + 1) * 256],
                )
            return
        if sq not in ost_tiles:
            ost_tiles[sq] = ost_pool.tile([128, 1024], dt.bfloat16, name="ost")
        ost = ost_tiles[sq]
        nc.vector.tensor_copy(out=ost[:, oc * 512:(oc + 1) * 512], in_=ps)
        if oc == 1:     # both halves copied: one DMA per 128-row block
            nc.sync.dma_start(
                out=outp.ap()[sq * 128:(sq + 1) * 128, :],
                in_=ost_tiles.pop(sq),
            )

    tsb_tiles = {}
    pend = []           # cross-unit PV trail: closures

    def pop_pend():
        if pend:
            pend.pop(0)()

    def emit_attn(qc, hp):
        """256-wide q chunk qc for heads h0=2*hp (PE rows 0:64) and h1=2*hp+1
        (rows 64:128)."""
        nkb = 2 * qc + 2
        q0 = QW * qc
        force_v(nkb - 1)    # vo_sb[kb<=nkb-1] must exist before its PV
        if qc >= 6:         # qt/kt chunk 3 must exist before these scores
            force_p3(hp)
        ctx_ps = psum_ctx.tile([128, 512], dt.float32, name="ctx")
        first_pv = [True]
        n_halves = 2 * ((nkb + 3) // 4)
        emitted = [0]

        def emit_pv(half, kbs, offs, ns, pt):
            for kb, off, n in zip(kbs, offs, ns):
                qoff = QW - n           # first valid q-col within the chunk
                for j in range(2):
                    if 128 * j < qoff:
                        continue        # sub-block entirely above diagonal
                    seg = off + 128 * j - qoff
                    blk = 65 * (2 * half + j)
                    last_kb = min(nkb - 1, 2 * qc + j)
                    nc.tensor.matmul(
                        ctx_ps[:, blk:blk + 65],
                        lhsT=pt[:, seg:seg + 128],
                        rhs=vo_sb[kb][:, 2 * hp + half, :],
                        start=first_pv[0],
                        stop=(half == 1 and kb == last_kb == 2 * qc + 1),
                        skip_group_check=True,
                    )
                    first_pv[0] = False
            emitted[0] += 1
            if emitted[0] == n_halves:
                emit_norm()

        def emit_norm():
            recip = small.tile([128, 4], dt.float32, name="recip")
            for b in range(4):
                nc.vector.reciprocal(
                    out=recip[:, b:b + 1],
                    in_=ctx_ps[:, 65 * b + 64:65 * b + 65])
            ts = [tsb_pool.tile([128, 128], dt.bfloat16, name="tsb")
                  for _ in range(2)]
            for half in range(2):
                for j in range(2):
                    blk = 65 * (2 * half + j)
                    nc.vector.tensor_scalar_mul(
                        out=ts[j][:, half * 64:(half + 1) * 64],
                        in0=ctx_ps[:, blk:blk + 64],
                        scalar1=recip[:, 2 * half + j:2 * half + j + 1],
                    )
            tsb_tiles[(qc, hp)] = ts
            fillers.append(
                [1, slot[0] + 2, 250, lambda: emit_transpose(qc, hp)])

        for c0 in range(0, nkb, 4):
            kbs = list(range(c0, min(c0 + 4, nkb)))
            ns = [QW - max(0, kb * 128 - q0) for kb in kbs]
            offs = [int(v) for v in np.cumsum([0] + ns[:-1])]
            ntot = offs[-1] + ns[-1]
            for half in range(2):
                p0 = half * 64
                slot[0] += 1
                emit_filler()   # before the scores: hides the stp-slot wait
                stp = psum_st.tile([128, 1024], dt.float32, name="stp")
                for kb, off, n in zip(kbs, offs, ns):
                    nc.tensor.matmul(
                        stp[:, off:off + n],
                        lhsT=kt_sb[hp][p0:p0 + 64, kb * 128:(kb + 1) * 128],
                        rhs=qt_sb[hp][p0:p0 + 64, q0 + QW - n:q0 + QW],
                        start=True,
                        stop=True,
                        tile_position=(p0, 0) if PACK_HEADS else None,
                    )
                pt = pt_pool.tile([128, 1024], dt.bfloat16, name="pt")
                nc.scalar.activation(
                    out=pt[:, :ntot], in_=stp[:, :ntot], func=F.Exp,
                    scale=0.125)
                for kb, off, n in zip(kbs, offs, ns):
                    qoff = QW - n
                    if kb == 2 * qc and qoff == 0:      # j=0 diagonal block
                        nc.vector.tensor_mul(
                            pt[:, off:off + 128], pt[:, off:off + 128], tri)
                    if kb == 2 * qc + 1:                # j=1 diagonal block
                        nc.vector.tensor_mul(
                            pt[:, off:off + 128], pt[:, off:off + 128], tri)
                pend.append(
                    lambda a=half, b=kbs, c=offs, d=ns, e=pt:
                    emit_pv(a, b, c, d, e))
                while len(pend) > PV_DEPTH:
                    pop_pend()

    # ---- emission schedule ----
    # Front: V for seq<1024 (DMA-paced), then proj chunk-0 pairs woven with
    # qc=0/1 attention (which only touches the first 512 cols of qt/kt).
    # Proj chunks c>=1 weave with qc=2/3; V for seq>=1024 becomes filler for
    # the exp-heavy late units.  attn(qc,hp) trails its proj pair by >=1 unit.
    emit_v_quarter(0)
    for st in range(4, 8):      # quarter 1 via psum_mm: per-st copy overlap,
        emit_v_st(st)           # and V q0's stp copies drain meanwhile
    emit_proj_pair(0, 0)
    emit_proj_pair(1, 0)
    emit_attn(0, 0)
    emit_proj_pair(2, 0)
    emit_attn(0, 1)
    emit_proj_pair(3, 0)
    emit_attn(0, 2)
    emit_attn(0, 3)
    for hp in range(4):
        emit_attn(1, hp)
    emit_proj_pair(0, 1)
    emit_proj_pair(1, 1)
    emit_attn(3, 0)
    emit_proj_pair(2, 1)
    emit_attn(3, 1)
    emit_proj_pair(3, 1)
    emit_attn(3, 2)
    emit_attn(3, 3)
    for st in range(8, 16):
        fn = lambda st=st: emit_v_st(st)
        fn.v_st = st
        fillers.append([0, 0, 1750, fn])
    emit_proj_pair(0, 2)
    emit_proj_pair(1, 2)
    emit_attn(4, 0)
    emit_proj_pair(2, 2)
    emit_attn(4, 1)
    emit_proj_pair(3, 2)
    emit_attn(4, 2)
    emit_attn(4, 3)
    for d in range(4):          # chunk-3 projections become late fillers:
        for half in range(2):   # the qc>=5 units are ACT-bound, this is free
            fn = lambda d=d, half=half: emit_proj_half(d, 3, half)
            fn.p3_d = d
            fillers.append([1, LATE0, 1750, fn])
    for hp in range(4):
        emit_attn(5, hp)
    for qc in (6, 7, 2):        # qc=2 last: smallest final dependency chain
        for hp in range(4):
            emit_attn(qc, hp)
    while pend:
        pop_pend()
    flush_fillers()

    if dbg is not None:
        for t in range(4):
            nc.sync.dma_start(out=dbg["qt"].ap()[t], in_=qt_sb[t])
            nc.sync.dma_start(out=dbg["kt"].ap()[t], in_=kt_sb[t])
            nc.sync.dma_start(out=dbg["cxt"].ap()[t], in_=cxt_sb[t])
        for t in range(16):
            nc.sync.dma_start(out=dbg["vo"].ap()[t], in_=vo_sb[t])

    return pools


def _build_nc():
    import concourse.tile as tile
    from concourse import bacc, mybir

    dt = mybir.dt
    nc = bacc.Bacc("TRN2", target_bir_lowering=False, debug=False,
                   num_devices=NCORES)
    allin = nc.dram_tensor("allin", [D, AIW], dt.bfloat16,
                           kind="ExternalInput")
    outp = nc.dram_tensor("outp", [S, D], dt.bfloat16, kind="ExternalOutput")

    with tile.TileContext(nc) as tc:
        pools = _build_body(tc, nc, mybir, allin, outp)
        pools.close()
    nc.compile()
    return nc


LAST_RESULTS = None


def kernel(batch, w_query, w_key, w_value, out_w, out_b):
    global LAST_RESULTS
    import os
    from concourse import bass_utils

    try:  # BASS_TRACE needs the axon NTFF hook; without it the run crashes
        from antenv.axon_hooks import get_axon_ntff_profile_hook  # noqa: F401
    except ImportError:
        os.environ.setdefault("BASS_NEVER_TRACE", "1")

    batch = np.asarray(batch, dtype=np.float32)
    w_query = np.asarray(w_query, dtype=np.float32)
    w_key = np.asarray(w_key, dtype=np.float32)
    w_value = np.asarray(w_value, dtype=np.float32)
    out_w = np.asarray(out_w, dtype=np.float32)
    out_b = np.asarray(out_b, dtype=np.float32)

    if "nc" not in _cache:
        _cache["nc"] = _build_nc()
    nc = _cache["nc"]

    xts = [np.ascontiguousarray(batch[b].T).astype(BF16) for b in range(B)]
    slc = [slice(g * GD, (g + 1) * GD) for g in range(2)]
    wqs = [w_query[:, s].astype(BF16) for s in slc]
    wks = [w_key[:, s].astype(BF16) for s in slc]
    wvs = [w_value[:, s].astype(BF16) for s in slc]
    ows = [out_w[s, :].astype(BF16) for s in slc]
    allins = {}
    for b in range(B):
        for g in range(2):
            ai = np.zeros((D, AIW), BF16)
            ai[:, 0:512] = wvs[g]
            ai[:, 512:1536] = xts[b][:, 0:1024]
            ai[:, 1536:2048] = wqs[g]
            ai[:, 2048:2560] = wks[g]
            ai[:, 2560:3584] = xts[b][:, 1024:2048]
            ai[0:GD, 3584:AIW] = ows[g]
            allins[(b, g)] = ai
    in_maps = []
    for c in range(NCORES):
        b, g = divmod(c, 2)
        in_maps.append({"allin": allins[(b, g)]})

    res = bass_utils.run_bass_kernel_spmd(
        nc, in_maps, core_ids=list(range(NCORES)),
    )
    LAST_RESULTS = res

    out = np.empty((B, S, D), np.float32)
    for b in range(B):
        out[b] = res.results[2 * b]["outp"].astype(np.float32) \
            + res.results[2 * b + 1]["outp"].astype(np.float32) \
            + out_b[None, :]
    return out


# revision 46
# speedup vs baseline: 1.0537x; 1.0537x over previous
"""Multi-head causal attention (B=4, S=2048, D=1024, 16 heads) on 8 TRN2 cores.

Sharding: core c -> (batch b = c//2, head-group g = c%2). Each core computes
8 heads of one batch element end-to-end (QKV proj, causal softmax attention,
out-proj rows for its head slice). Host sums the two head-group partials per
batch and adds the output bias.

Per-core pipeline (all matmuls contraction-on-partitions, bf16 in / f32 psum):
  V first (quarter 0 din-major so each (wv,xT-chunk) DMA pair unblocks work;
  later quarters group-major so psum->sbuf copies overlap the remaining
  matmuls), then QT/KT[dtile] = (x @ w)^T with attention interleaved.
  attention per (256-wide q-chunk qc, head-pair hp):
    ST[k,q] = KT.T @ QT into a [128,1024] psum (<=4 k-blocks per exp chunk)
    PT = exp(ST/8) bf16; tri-mask on diagonal 128-col blocks
    ctxT[q128, 65] += PT-block.T @ [V_h|1]   (N=65 matmuls: cost-model cheap;
      col 64 = softmax denominator). One start=True per ctx psum bank (PSUM
      pending-zero covers every block's first write; a second start would
      wipe sibling blocks).  PV trails the exp by 2 chunk-halves ACROSS
      attention units so the PE never sits behind the ACT engine.
    normalize per-partition: ctxT[:, :64] * recip(ctxT[:, 64]) -> t_sb bf16
  t_sb tiles are PE-transposed (via identity) back to cxt[2-head-hd, q].
  out[seq128, 512] = cxt.T @ ow streamed to DRAM per 128-row block.
  Transposes + out-proj pieces are "fillers" popped between score chunks;
  the late phase runs qc descending so out-proj work for early rows is
  available to fill the exp-heavy qc=7/6 units.
"""

import numpy as np
import ml_dtypes

B, S, D = 4, 2048, 1024
H_TOT = 16
HD = 64
NCORES = 8
GH = 8          # heads per core
GD = GH * HD    # 512: dout slice per core
NKB = S // 128  # 16 k-blocks
QW = 256        # q-chunk width
NQC = S // QW   # 8 q-chunks
BF16 = ml_dtypes.bfloat16

PACK_HEADS = True   # pack 2 heads' score matmuls into PE row groups
PV_DEPTH = 3        # chunk-halves the PV trails behind the exp
AIW = 512 * 3 + S + 1024   # packed input width: [wv | wq | wk | xT | ow]

_cache = {}


def _build_body(tc, nc, mybir, allin, outp, dbg=None):
    from concourse.masks import make_upper_triangular, make_identity
    import contextlib

    dt = mybir.dt
    F = mybir.ActivationFunctionType

    pools = contextlib.ExitStack()
    tc_pool = lambda **kw: pools.enter_context(tc.tile_pool(**kw))

    singles = tc_pool(name="singles", bufs=1)
    pt_pool = tc_pool(name="pt", bufs=6)
    tsb_pool = tc_pool(name="tsb", bufs=8)
    small = tc_pool(name="small", bufs=4)
    ost_pool = tc_pool(name="ost", bufs=4)
    psum_st = tc_pool(name="psum_st", bufs=2, space="PSUM")
    psum_ctx = tc_pool(name="psum_ctx", bufs=2, space="PSUM")
    psum_mm = tc_pool(name="psum_mm", bufs=2, space="PSUM")

    # ---- persistent SBUF tensors.  All per-din inputs live in ONE packed
    # tile per din ([wv | xtA | wq | wk | xtB | ow]) so the whole input load
    # is 24 large DMAs: HWDGE descriptor generation (~0.6us per DMA, one
    # shared unit) would otherwise serialize ~60 small DMAs for ~38us.  The
    # V-projection operands come first so the PE starts ~2us in.
    allin_sb = [singles.tile([128, AIW], dt.bfloat16, name=f"ai{t}")
                for t in range(8)]
    wv_sb = [t[:, 0:512] for t in allin_sb]
    wq_sb = [t[:, 1536:2048] for t in allin_sb]
    wk_sb = [t[:, 2048:2560] for t in allin_sb]
    ow_sb = [allin_sb[t][:, 3584:AIW] for t in range(4)]

    def xts(din, s0, s1):
        """xT[din] seq-col slice [s0:s1); must not cross the 1024 boundary."""
        if s1 <= 1024:
            return allin_sb[din][:, 512 + s0:512 + s1]
        assert s0 >= 1024
        return allin_sb[din][:, 2560 + s0 - 1024:2560 + s1 - 1024]

    xT_sb = None  # accessed via xts()
    qt_sb = [singles.tile([128, S], dt.bfloat16, name=f"qt{t}")
             for t in range(4)]                              # 2 heads / dtile
    kt_sb = [singles.tile([128, S], dt.bfloat16, name=f"kt{t}")
             for t in range(4)]
    vo_sb = [singles.tile([128, GH, 65], dt.bfloat16, name=f"vo{t}")
             for t in range(NKB)]                            # [V_h | ones]
    cxt_sb = [singles.tile([128, S], dt.bfloat16, name=f"cxt{t}")
              for t in range(4)]                             # ctx, 2 heads/tile
    tri = singles.tile([128, 128], dt.bfloat16)              # keep k<=q
    ident = singles.tile([128, 128], dt.bfloat16)

    make_upper_triangular(nc, tri, val=1.0, diag=True)
    make_identity(nc, ident)
    for t in range(NKB):
        nc.vector.memset(vo_sb[t][:, :, 64:65], 1.0)

    # ---- input DMAs: 3 pieces per din on the SP HWDGE queue.  Piece A
    # (wv + xT first half) feeds the V projections, B (wq+wk) the Q/K
    # projections, C the rest.  The ACT queue carries nothing, so the ACT
    # sequencer is free for exp dispatch.
    ai_r = allin.ap().rearrange("(t p) n -> p t n", p=128)
    for lo, hi in ((0, 1536), (1536, 2560), (2560, AIW)):
        for din in range(8):
            nc.sync.dma_start(out=allin_sb[din][:, lo:hi],
                              in_=ai_r[:, din, lo:hi])

    def emit_v_quarter(q4):
        """V proj for seq blocks 4*q4..4*q4+3.  Quarter 0 is DMA-paced:
        din-major so each arriving (wv,xT) pair feeds 4 matmuls.  Later
        quarters are group-major so each group's copy overlaps the rest."""
        pst = [psum_st.tile([128, 1024], dt.float32, name="stp")
               for _ in range(2)]
        pss = [pst[0][:, 0:512], pst[0][:, 512:1024],
               pst[1][:, 0:512], pst[1][:, 512:1024]]

        def mm(i, din):
            st = 4 * q4 + i
            nc.tensor.matmul(
                pss[i],
                lhsT=xts(din, st * 128, (st + 1) * 128),
                rhs=wv_sb[din],
                start=(din == 0),
                stop=(din == 7),
            )

        def cp(i):
            nc.vector.tensor_copy(
                out=vo_sb[4 * q4 + i][:, :, 0:64],
                in_=pss[i].rearrange("p (h d) -> p h d", h=GH),
            )

        if q4 == 0:
            for din in range(8):
                for i in range(4):
                    mm(i, din)
            for i in range(4):
                cp(i)
        else:
            for i in range(4):
                for din in range(8):
                    mm(i, din)
                cp(i)

    def emit_v_st(st):
        """One V seq-block as late-phase filler work (psum_mm based)."""
        ps = psum_mm.tile([128, 512], dt.float32, name="mmps")
        for din in range(8):
            nc.tensor.matmul(
                ps,
                lhsT=xts(din, st * 128, (st + 1) * 128),
                rhs=wv_sb[din],
                start=(din == 0),
                stop=(din == 7),
            )
        nc.vector.tensor_copy(
            out=vo_sb[st][:, :, 0:64],
            in_=ps.rearrange("p (h d) -> p h d", h=GH),
        )
        v_emitted.add(st)

    def emit_proj_pair(d, c):
        """Q and K projection of dtile d, 512-col seq chunk c (one stp tile)."""
        pst = psum_st.tile([128, 1024], dt.float32, name="stp")
        for half, (w_sb, t_sb) in enumerate(((wq_sb, qt_sb), (wk_sb, kt_sb))):
            ps = pst[:, half * 512:(half + 1) * 512]
            for din in range(8):
                nc.tensor.matmul(
                    ps,
                    lhsT=w_sb[din][:, d * 128:(d + 1) * 128],
                    rhs=xts(din, c * 512, (c + 1) * 512),
                    start=(din == 0),
                    stop=(din == 7),
                )
            nc.vector.tensor_copy(
                out=t_sb[d][:, c * 512:(c + 1) * 512], in_=ps)

    def emit_proj_half(d, c, half):
        """Half a proj pair (Q or K) as late filler work (psum_mm based)."""
        w_sb, t_sb = ((wq_sb, qt_sb), (wk_sb, kt_sb))[half]
        ps = psum_mm.tile([128, 512], dt.float32, name="mmps")
        for din in range(8):
            nc.tensor.matmul(
                ps,
                lhsT=w_sb[din][:, d * 128:(d + 1) * 128],
                rhs=xts(din, c * 512, (c + 1) * 512),
                start=(din == 0),
                stop=(din == 7),
            )
        nc.vector.tensor_copy(
            out=t_sb[d][:, c * 512:(c + 1) * 512], in_=ps)

    # ---- filler machinery: small PE work units popped between score chunks.
    # Pops are RATIONED to ~the per-chunk-half ACT-vs-PE deficit so the
    # supply lasts through the final (most exp-heavy) units.
    LATE0 = 56          # chunk-half index where the ACT-bound phase begins
    DEF_NS = 800        # filler allowance earned per chunk-half
    slot = [0]          # global chunk-half counter
    allow = [0.0]       # accumulated filler allowance (ns)
    fillers = []        # [prio, ready_slot, cost_ns, fn] entries
    tr_done = [0] * NQC
    v_emitted = set(range(8))   # V seq-blocks already materialized

    def emit_filler():
        best = None
        for i, (prio, rdy, cost, fn) in enumerate(fillers):
            if rdy <= slot[0] and (best is None or prio < fillers[best][0]):
                best = i
        if best is not None:
            fillers.pop(best)[3]()

    def force_v(st_max):
        """Emit any still-queued V fillers with st <= st_max right now."""
        for i in range(len(fillers) - 1, -1, -1):
            if getattr(fillers[i][3], "v_st", 99) <= st_max:
                fillers.pop(i)[3]()

    def force_p3(d):
        """Emit any still-queued chunk-3 proj fillers for dtile d."""
        for i in range(len(fillers) - 1, -1, -1):
            if getattr(fillers[i][3], "p3_d", -1) == d:
                fillers.pop(i)[3]()

    def flush_fillers():
        while fillers:
            best = None
            for i, ent in enumerate(fillers):
                if best is None or ent[0] < fillers[best][0]:
                    best = i
            fillers.pop(best)[3]()

    def emit_transpose(qc, hp):
        tps = psum_mm.tile([128, 256], dt.bfloat16, name="mmps")
        ts0, ts1 = tsb_tiles.pop((qc, hp))
        nc.tensor.transpose(tps[:, 0:128], ts0, ident)
        nc.tensor.transpose(tps[:, 128:256], ts1, ident)
        nc.vector.tensor_copy(
            out=cxt_sb[hp][:, qc * QW:(qc + 1) * QW], in_=tps)
        tr_done[qc] += 1
        if tr_done[qc] == 4:
            for sq in (2 * qc, 2 * qc + 1):
                for oc in range(2):
                    fillers.append(
                        [2, max(slot[0] + 2, LATE0), 900,
                         lambda sq=sq, oc=oc: emit_outproj_piece(sq, oc)])

    ost_tiles = {}

    def emit_outproj_piece(sq, oc):
        ps = psum_mm.tile([128, 512], dt.float32, name="mmps")
        for dvt in range(4):
            nc.tensor.matmul(
                ps,
                lhsT=cxt_sb[dvt][:, sq * 128:(sq + 1) * 128],
                rhs=ow_sb[dvt][:, oc * 512:(oc + 1) * 512],
                start=(dvt == 0),
                stop=(dvt == 3),
            )
        if sq // 2 == 2:    # final range (qc=2 is last): fine-grained drain
            ost = ost_pool.tile([128, 1024], dt.bfloat16, name="ost")
            for i in range(2):
                nc.vector.tensor_copy(
                    out=ost[:, i * 256:(i + 1) * 256],
                    in_=ps[:, i * 256:(i + 1) * 256])
                nc.sync.dma_start(
                    out=outp.ap()[sq * 128:(sq + 1) * 128,
                                  oc * 512 + i * 256:oc * 512 + (i + 1) * 256],
                    in_=ost[:, i * 256:(i + 1) * 256],
                )
            return
        if sq not in ost_tiles:
            ost_tiles[sq] = ost_pool.tile([128, 1024], dt.bfloat16, name="ost")
        ost = ost_tiles[sq]
        nc.vector.tensor_copy(out=ost[:, oc * 512:(oc + 1) * 512], in_=ps)
        if oc == 1:     # both halves copied: one DMA per 128-row block
            nc.sync.dma_start(
                out=outp.ap()[sq * 128:(sq + 1) * 128, :],
                in_=ost_tiles.pop(sq),
            )

    tsb_tiles = {}
    pend = []           # cross-unit PV trail: closures

    def pop_pend():
        if pend:
            pend.pop(0)()

    def emit_attn(qc, hp):
        """256-wide q chunk qc for heads h0=2*hp (PE rows 0:64) and h1=2*hp+1
        (rows 64:128)."""
        nkb = 2 * qc + 2
        q0 = QW * qc
        force_v(nkb - 1)    # vo_sb[kb<=nkb-1] must exist before its PV
        if qc >= 6:         # qt/kt chunk 3 must exist before these scores
            force_p3(hp)
        ctx_ps = psum_ctx.tile([128, 512], dt.float32, name="ctx")
        first_pv = [True]
        n_halves = 2 * ((nkb + 3) // 4)
        emitted = [0]

        def emit_pv(half, kbs, offs, ns, pt):
            for kb, off, n in zip(kbs, offs, ns):
                qoff = QW - n           # first valid q-col within the chunk
                for j in range(2):
                    if 128 * j < qoff:
                        continue        # sub-block entirely above diagonal
                    seg = off + 128 * j - qoff
                    blk = 65 * (2 * half + j)
                    last_kb = min(nkb - 1, 2 * qc + j)
                    nc.tensor.matmul(
                        ctx_ps[:, blk:blk + 65],
                        lhsT=pt[:, seg:seg + 128],
                        rhs=vo_sb[kb][:, 2 * hp + half, :],
                        start=first_pv[0],
                        stop=(half == 1 and kb == last_kb == 2 * qc + 1),
                        skip_group_check=True,
                    )
                    first_pv[0] = False
            emitted[0] += 1
            if emitted[0] == n_halves:
                emit_norm()

        def emit_norm():
            recip = small.tile([128, 4], dt.float32, name="recip")
            for b in range(4):
                nc.vector.reciprocal(
                    out=recip[:, b:b + 1],
                    in_=ctx_ps[:, 65 * b + 64:65 * b + 65])
            ts = [tsb_pool.tile([128, 128], dt.bfloat16, name="tsb")
                  for _ in range(2)]
            for half in range(2):
                for j in range(2):
                    blk = 65 * (2 * half + j)
                    nc.vector.tensor_scalar_mul(
                        out=ts[j][:, half * 64:(half + 1) * 64],
                        in0=ctx_ps[:, blk:blk + 64],
                        scalar1=recip[:, 2 * half + j:2 * half + j + 1],
                    )
            tsb_tiles[(qc, hp)] = ts
            fillers.append(
                [1, slot[0] + 2, 250, lambda: emit_transpose(qc, hp)])

        for c0 in range(0, nkb, 4):
            kbs = list(range(c0, min(c0 + 4, nkb)))
            ns = [QW - max(0, kb * 128 - q0) for kb in kbs]
            offs = [int(v) for v in np.cumsum([0] + ns[:-1])]
            ntot = offs[-1] + ns[-1]
            for half in range(2):
                p0 = half * 64
                slot[0] += 1
                emit_filler()   # before the scores: hides the stp-slot wait
                stp = psum_st.tile([128, 1024], dt.float32, name="stp")
                for kb, off, n in zip(kbs, offs, ns):
                    nc.tensor.matmul(
                        stp[:, off:off + n],
                        lhsT=kt_sb[hp][p0:p0 + 64, kb * 128:(kb + 1) * 128],
                        rhs=qt_sb[hp][p0:p0 + 64, q0 + QW - n:q0 + QW],
                        start=True,
                        stop=True,
                        tile_position=(p0, 0) if PACK_HEADS else None,
                    )
                pt = pt_pool.tile([128, 1024], dt.bfloat16, name="pt")
                nc.scalar.activation(
                    out=pt[:, :ntot], in_=stp[:, :ntot], func=F.Exp,
                    scale=0.125)
                for kb, off, n in zip(kbs, offs, ns):
                    qoff = QW - n
                    if kb == 2 * qc and qoff == 0:      # j=0 diagonal block
                        nc.vector.tensor_mul(
                            pt[:, off:off + 128], pt[:, off:off + 128], tri)
                    if kb == 2 * qc + 1:                # j=1 diagonal block
                        nc.vector.tensor_mul(
                            pt[:, off:off + 128], pt[:, off:off + 128], tri)
                pend.append(
                    lambda a=half, b=kbs, c=offs, d=ns, e=pt:
                    emit_pv(a, b, c, d, e))
                while len(pend) > PV_DEPTH:
                    pop_pend()

    # ---- emission schedule ----
    # Front: V for seq<1024 (DMA-paced), then proj chunk-0 pairs woven with
    # qc=0/1 attention (which only touches the first 512 cols of qt/kt).
    # Proj chunks c>=1 weave with qc=2/3; V for seq>=1024 becomes filler for
    # the exp-heavy late units.  attn(qc,hp) trails its proj pair by >=1 unit.
    emit_v_quarter(0)
    for st in range(4, 8):      # quarter 1 via psum_mm: per-st copy overlap,
        emit_v_st(st)           # and V q0's stp copies drain meanwhile
    emit_proj_pair(0, 0)
    emit_proj_pair(1, 0)
    emit_attn(0, 0)
    emit_proj_pair(2, 0)
    emit_attn(0, 1)
    emit_proj_pair(3, 0)
    emit_attn(0, 2)
    emit_attn(0, 3)
    for hp in range(4):
        emit_attn(1, hp)
    emit_proj_pair(0, 1)
    emit_proj_pair(1, 1)
    emit_attn(3, 0)
    emit_proj_pair(2, 1)
    emit_attn(3, 1)
    emit_proj_pair(3, 1)
    emit_attn(3, 2)
    emit_attn(3, 3)
    for st in range(8, 16):
        fn = lambda st=st: emit_v_st(st)
        fn.v_st = st
        fillers.append([0, 0, 1750, fn])
    emit_proj_pair(0, 2)
    emit_proj_pair(1, 2)
    emit_attn(4, 0)
    emit_proj_pair(2, 2)
    emit_attn(4, 1)
    emit_proj_pair(3, 2)
    emit_attn(4, 2)
    emit_attn(4, 3)
    for d in range(4):          # chunk-3 projections become late fillers:
        for half in range(2):   # the qc>=5 units are ACT-bound, this is free
            fn = lambda d=d, half=half: emit_proj_half(d, 3, half)
            fn.p3_d = d
            fillers.append([1, LATE0, 1750, fn])
    for hp in range(4):
        emit_attn(5, hp)
    for qc in (6, 7, 2):        # qc=2 last: smallest final dependency chain
        for hp in range(4):
            emit_attn(qc, hp)
    while pend:
        pop_pend()
    flush_fillers()

    if dbg is not None:
        for t in range(4):
            nc.sync.dma_start(out=dbg["qt"].ap()[t], in_=qt_sb[t])
            nc.sync.dma_start(out=dbg["kt"].ap()[t], in_=kt_sb[t])
            nc.sync.dma_start(out=dbg["cxt"].ap()[t], in_=cxt_sb[t])
        for t in range(16):
            nc.sync.dma_start(out=dbg["vo"].ap()[t], in_=vo_sb[t])

    return pools


def _build_nc():
    import concourse.tile as tile
    from concourse import bacc, mybir

    dt = mybir.dt
    nc = bacc.Bacc("TRN2", target_bir_lowering=False, debug=False,
                   num_devices=NCORES)
    allin = nc.dram_tensor("allin", [D, AIW], dt.bfloat16,
                           kind="ExternalInput")
    outp = nc.dram_tensor("outp", [S, D], dt.bfloat16, kind="ExternalOutput")

    with tile.TileContext(nc) as tc:
        pools = _build_body(tc, nc, mybir, allin, outp)
        pools.close()
    nc.compile()
    return nc


LAST_RESULTS = None


def kernel(batch, w_query, w_key, w_value, out_w, out_b):
    global LAST_RESULTS
    import os
    from concourse import bass_utils

    try:  # BASS_TRACE needs the axon NTFF hook; without it the run crashes
        from antenv.axon_hooks import get_axon_ntff_profile_hook  # noqa: F401
    except ImportError:
        os.environ.setdefault("BASS_NEVER_TRACE", "1")

    batch = np.asarray(batch, dtype=np.float32)
    w_query = np.asarray(w_query, dtype=np.float32)
    w_key = np.asarray(w_key, dtype=np.float32)
    w_value = np.asarray(w_value, dtype=np.float32)
    out_w = np.asarray(out_w, dtype=np.float32)
    out_b = np.asarray(out_b, dtype=np.float32)

    if "nc" not in _cache:
        _cache["nc"] = _build_nc()
    nc = _cache["nc"]

    xts = [np.ascontiguousarray(batch[b].T).astype(BF16) for b in range(B)]
    slc = [slice(g * GD, (g + 1) * GD) for g in range(2)]
    wqs = [w_query[:, s].astype(BF16) for s in slc]
    wks = [w_key[:, s].astype(BF16) for s in slc]
    wvs = [w_value[:, s].astype(BF16) for s in slc]
    ows = [out_w[s, :].astype(BF16) for s in slc]
    allins = {}
    for b in range(B):
        for g in range(2):
            ai = np.zeros((D, AIW), BF16)
            ai[:, 0:512] = wvs[g]
            ai[:, 512:1536] = xts[b][:, 0:1024]
            ai[:, 1536:2048] = wqs[g]
            ai[:, 2048:2560] = wks[g]
            ai[:, 2560:3584] = xts[b][:, 1024:2048]
            ai[0:GD, 3584:AIW] = ows[g]
            allins[(b, g)] = ai
    in_maps = []
    for c in range(NCORES):
        b, g = divmod(c, 2)
        in_maps.append({"allin": allins[(b, g)]})

    res = bass_utils.run_bass_kernel_spmd(
        nc, in_maps, core_ids=list(range(NCORES)),
    )
    LAST_RESULTS = res

    out = np.empty((B, S, D), np.float32)
    for b in range(B):
        out[b] = res.results[2 * b]["outp"].astype(np.float32) \
            + res.results[2 * b + 1]["outp"].astype(np.float32) \
            + out_b[None, :]
    return out
